# revision 17
# baseline (speedup 1.0000x reference)
"""Multi-head attention (B=2, S=2048, D=1024, H=16) on 8 Trainium2 cores.

Sharding: core c handles batch c//4 and head-group c%4 (4 heads x dk 64).
Q/K/V projection weights are column-split by head group on the host; the
output projection is split by OUTPUT column: core c computes all 2048
tokens x its 256 output columns, so each core consumes the full gathered
concat but no final collective or dynamic slice is needed.

Attention runs in 4 chunks of 512 query tokens x 4 heads.  Scores stay in
[k, q] orientation; the PV product streams exp-scores against a stationary
V slice plus a ones column, giving [dk+1, q] with the softmax denominator
in row dk.  Normalization: DVE fast-reciprocal on the denominator row ->
gpsimd partition-broadcast -> fused multiply.  Scores of block i+1
interleave with the PV matmuls of block i so PE and ACT stay busy.

As soon as a chunk's 4 heads are normalized, an AllGather ships the
[256, 512] per-head outputs inside each 4-core batch group; the output
projection for that chunk is interleaved into a later chunk's attention
stream so the collective latency is hidden.  Only the last chunk's
AllGather + projection are exposed (~25us tail).
"""

import numpy as np
import ml_dtypes

import concourse.bass as bass
import concourse.tile as tile
from concourse import bacc, mybir
from concourse.bass_utils import run_bass_kernel_spmd

BF16 = mybir.dt.bfloat16
F32 = mybir.dt.float32
NPBF16 = ml_dtypes.bfloat16

B, S, D, H = 2, 2048, 1024, 16
DK = 64
DK1 = DK + 1
N_CORES = 8
HPC = 4               # heads per core
FEAT = HPC * DK       # 256 projected features per core
VW = HPC * DK1        # 260: v with a ones column per head
OCOL = 256            # output columns per core
TOKC = 1024           # token chunk for projections
QCH = 512             # q chunk for attention (= AllGather granularity)
NCH = S // QCH        # 4 chunks
NKT = S // 128        # 16 k tiles
NKC = D // 128        # 8 contraction chunks

_CACHE = {}


def _build_program():
    if "nc" in _CACHE:
        return _CACHE["nc"]

    nc = bacc.Bacc("TRN2", target_bir_lowering=False, debug=False,
                   num_devices=N_CORES)

    xq = nc.declare_dram_parameter("xq", [D, S], BF16, isOutput=False)
    xk = nc.declare_dram_parameter("xk", [D, S], BF16, isOutput=False)
    xv = nc.declare_dram_parameter("xv", [D, S], BF16, isOutput=False)
    wq = nc.declare_dram_parameter("wq", [D, FEAT], BF16, isOutput=False)
    wk = nc.declare_dram_parameter("wk", [D, FEAT], BF16, isOutput=False)
    wv = nc.declare_dram_parameter("wv", [D, VW], BF16, isOutput=False)
    wo = nc.declare_dram_parameter("wo", [D, OCOL], BF16, isOutput=False)
    bq = nc.declare_dram_parameter("bq", [128, 2], F32, isOutput=False)
    bk = nc.declare_dram_parameter("bk", [128, 2], F32, isOutput=False)
    bv = nc.declare_dram_parameter("bv", [1, VW], BF16, isOutput=False)
    bo = nc.declare_dram_parameter("bo", [1, OCOL], BF16, isOutput=False)
    out = nc.declare_dram_parameter("out", [S, OCOL], BF16, isOutput=True)
    dbg = {}

    with tile.TileContext(nc) as tc:
        with (
            tc.tile_pool(name="w", bufs=1) as wpool,
            tc.tile_pool(name="x", bufs=26) as xpool,
            tc.tile_pool(name="qk", bufs=1) as qkpool,
            tc.tile_pool(name="vp", bufs=1) as vpool,
            tc.tile_pool(name="sct", bufs=18) as sctpool,
            tc.tile_pool(name="nm", bufs=2) as nmpool,
            tc.tile_pool(name="cat", bufs=16) as catpool,
            tc.tile_pool(name="fo", bufs=3) as fopool,
            tc.tile_pool(name="ps_a", bufs=3, space="PSUM") as ps_a,
            tc.tile_pool(name="ps_pv", bufs=1, space="PSUM") as ps_pv,
            tc.tile_pool(name="dram", bufs=1, space="DRAM") as dram,
        ):
            _emit(nc, wpool, xpool, qkpool, vpool, sctpool, nmpool,
                  catpool, fopool, ps_a, ps_pv, dram,
                  xq, xk, xv, wq, wk, wv, wo, bq, bk, bv, bo, out, dbg)

    nc.compile()
    _CACHE["nc"] = nc
    return nc


def _emit(nc, wpool, xpool, qkpool, vpool, sctpool, nmpool, catpool,
          fopool, ps_a, ps_pv, dram,
          xq, xk, xv, wq, wk, wv, wo, bq, bk, bv, bo, out, dbg={}):
    MUL = mybir.AluOpType.mult
    EXPF = mybir.ActivationFunctionType.Exp
    IDF = mybir.ActivationFunctionType.Identity

    ones1 = wpool.tile([1, 128], BF16, tag="ones")
    nc.vector.memset(ones1[:], 1.0)

    # DMA issue engines for bulk input loads (round-robin: the Sync engine
    # alone issues descriptors at ~600ns each, which gates phase 1).  Only
    # SP and Activation are hardware-DGE engines; gpsimd DMA goes through
    # the software-DGE ring and corrupts data in this flow.
    dmae = [nc.sync, nc.scalar]
    NE = len(dmae)

    wk_sb = []
    for kc in range(NKC):
        t = wpool.tile([128, FEAT], BF16, tag=f"wk{kc}")
        dmae[kc % NE].dma_start(t[:], wk[bass.ts(kc, 128), :])
        wk_sb.append(t)
    bk_sb = wpool.tile([128, 2], F32, tag="bk")
    nc.sync.dma_start(bk_sb[:], bk[:])

    qh_sb = [qkpool.tile([128, S], BF16, tag=f"qh{m}", name=f"qh{m}")
             for m in range(2)]
    kh_sb = [qkpool.tile([128, S], BF16, tag=f"kh{m}", name=f"kh{m}")
             for m in range(2)]
    v_sb = [vpool.tile([128, VW], BF16, tag=f"v{j}", name=f"v{j}")
            for j in range(NKT)]

    def load_x(src, t0):
        tiles = []
        for kc in range(NKC):
            t = xpool.tile([128, TOKC], BF16, tag="xt")
            dmae[kc % NE].dma_start(t[:], src[bass.ts(kc, 128),
                                              bass.ts(t0, TOKC)])
            tiles.append(t)
        return tiles

    def qk_group(w_sb, x_t, b_sb, dst, t0, m):
        ps = ps_a.tile([128, TOKC], F32, tag="a")
        for kc in range(NKC):
            for u in range(TOKC // 512):
                nc.tensor.matmul(
                    ps[:, bass.ts(u, 512)],
                    w_sb[kc][:, bass.ts(m, 128)],
                    x_t[kc][:, bass.ts(u, 512)],
                    start=(kc == 0), stop=(kc == NKC - 1),
                )
        nc.vector.tensor_scalar_add(dst[m][:, bass.ts(t0, TOKC)], ps[:],
                                    b_sb[:, m:m + 1])

    # ---- K projection (scores need the full kh) ------------------
    xk_ts = [load_x(xk, t0) for t0 in range(S // TOKC)]
    for t0 in range(S // TOKC):
        for m in range(2):
            qk_group(wk_sb, xk_ts[t0], bk_sb, kh_sb, t0, m)

    wq_sb = []
    for kc in range(NKC):
        t = wpool.tile([128, FEAT], BF16, tag=f"wq{kc}")
        dmae[kc % NE].dma_start(t[:], wq[bass.ts(kc, 128), :])
        wq_sb.append(t)
    bq_sb = wpool.tile([128, 2], F32, tag="bq")
    nc.sync.dma_start(bq_sb[:], bq[:])
    xq_t0 = load_x(xq, 0)
    for m in range(2):
        qk_group(wq_sb, xq_t0, bq_sb, qh_sb, 0, m)

    # ---- V weights + all remaining input loads -------------------
    # Issued now, before any exp lands on the scalar queue, so the
    # attention stream never waits on DMA descriptor issue.
    wv_sb = []
    for kc in range(NKC):
        t = wpool.tile([128, VW], BF16, tag=f"wv{kc}")
        dmae[kc % NE].dma_start(t[:], wv[bass.ts(kc, 128), :])
        wv_sb.append(t)
    bv_sb = wpool.tile([1, VW], BF16, tag="bv")
    nc.sync.dma_start(bv_sb[:], bv[:])
    xv_ts = [load_x(xv, t0) for t0 in range(S // TOKC)]
    xq_t1 = load_x(xq, 1)

    def v_group(t0, j):
        ps = ps_a.tile([128, VW], F32, tag="a")
        for kc in range(NKC):
            nc.tensor.matmul(
                ps[:], xv_ts[t0][kc][:, bass.ts(j, 128)], wv_sb[kc][:],
                start=(kc == 0), stop=False,
            )
        nc.tensor.matmul(ps[:], ones1[:], bv_sb[:], start=False, stop=True)
        nc.vector.tensor_copy(v_sb[t0 * (TOKC // 128) + j][:], ps[:])

    # wo + bo requested now: the 0.5 MB load drains during attention.
    wo_sb = []
    for kc in range(NKC):
        t = wpool.tile([128, OCOL], BF16, tag=f"wo{kc}")
        dmae[kc % NE].dma_start(t[:], wo[bass.ts(kc, 128), :])
        wo_sb.append(t)
    bo_sb = wpool.tile([1, OCOL], BF16, tag="bo")
    nc.sync.dma_start(bo_sb[:], bo[:])

    # ---- phase 2/3: attention + chunked AllGather + out proj -----
    # Attention runs in 2 q-blocks of 1024 x 4 heads (the baseline shape,
    # which paces best under the power throttle).  AllGathers fire per
    # 512-token chunk as soon as its heads are normalized; chunk 3 ships
    # in two half-gathers (heads 01 early, heads 23 at the end).  The
    # column-split output projections all run after attention, where the
    # ACT engine is quiet and the PE runs unthrottled; their collectives
    # are complete by then, so there is no dead zone.
    QB = 2 * QCH          # 1024-token attention block
    NQB = S // QB         # 2 blocks
    ag_in = [dram.tile([FEAT, 2 * QCH], BF16, tag=f"agi{b}", name=f"agi{b}")
             for b in range(NQB)]
    ag_out = [dram.tile([4 * FEAT, 2 * QCH], BF16, tag=f"ago{b}",
                        name=f"ago{b}") for b in range(NQB)]

    def emit_ag(eng, ins_ap, outs_ap):
        eng.collective_compute(
            "AllGather", mybir.AluOpType.bypass,
            replica_groups=[[0, 1, 2, 3], [4, 5, 6, 7]],
            ins=[ins_ap.opt()],
            outs=[outs_ap.opt()],
        )

    def norm_and_out(pv, h, qb):
        pvs = nmpool.tile([DK1, QB], F32, tag="pvs")
        nc.vector.tensor_copy(pvs[:], pv[:])
        drow = nmpool.tile([1, QB], BF16, tag="drow")
        nc.vector.tensor_copy(drow[:], pvs[DK:DK1, :])
        # Broadcast the denominator row via a rank-1 PE matmul instead of
        # gpsimd partition_broadcast, keeping the gpsimd queue free for the
        # AllGathers (its queue blocks while a collective is in flight).
        psb = ps_a.tile([DK, QB], F32, tag="a", name="psb")
        for u in range(QB // 512):
            nc.tensor.matmul(psb[:, bass.ts(u, 512)], ones1[0:1, 0:DK],
                             drow[:, bass.ts(u, 512)], start=True, stop=True)
        dbs = nmpool.tile([DK, QB], F32, tag="db")
        nc.vector.tensor_copy(dbs[:], psb[:])
        rb = nmpool.tile([DK, QB], F32, tag="rb")
        nc.vector.reciprocal_approx_fast(rb[:], dbs[:])
        onrm = nmpool.tile([DK, QB], BF16, tag="onrm")
        nc.vector.scalar_tensor_tensor(onrm[:], pvs[0:DK, :], 1.0, rb[:],
                                       MUL, MUL)
        nc.sync.dma_start(ag_in[qb][h * DK:(h + 1) * DK, :], onrm[:])
        if h == HPC - 1:
            emit_ag(nc.gpsimd, ag_in[qb][:], ag_out[qb][:])

    def out_proj(c):
        qb, u = c // 2, c % 2
        cat = []
        for kc in range(NKC):
            t = catpool.tile([128, QCH], BF16, tag="cat")
            dmae[kc % NE].dma_start(
                t[:], ag_out[qb][bass.ts(kc, 128), bass.ts(u, QCH)])
            cat.append(t)
        for qt in range(QCH // 128):
            ps = ps_a.tile([128, OCOL], F32, tag="a", name="po")
            nc.tensor.matmul(ps[:], ones1[:, 0:128], bo_sb[:],
                             start=True, stop=False)
            for kc in range(NKC):
                nc.tensor.matmul(
                    ps[:],
                    cat[kc][:, bass.ts(qt, 128)],
                    wo_sb[kc][:],
                    start=False, stop=(kc == NKC - 1),
                )
            fo = fopool.tile([128, OCOL], BF16, tag="fo")
            nc.scalar.activation(fo[:], ps[:], IDF)
            dmae[qt % NE].dma_start(
                out[bass.ts(c * (QCH // 128) + qt, 128), :], fo[:])

    blocks = [(qb, h) for qb in range(NQB) for h in range(HPC)]
    last = len(blocks) - 1
    prev = None
    for bi, (qb, h) in enumerate(blocks):
        if bi == 1:
            # V projection and the second Q-projection half run here, after
            # block (0,0)'s scores: the exp stream starts ~40us earlier
            # while the PE chews through these during block (0,0)'s exps.
            for t0 in range(S // TOKC):
                for j in range(TOKC // 128):
                    v_group(t0, j)
            for m in range(2):
                qk_group(wq_sb, xq_t1, bq_sb, qh_sb, 1, m)
        ht, hr = h // 2, (h % 2) * 64
        q0 = qb * QB
        if bi == last:
            pv = ps_a.tile([DK1, QB], F32, tag="a", name="pv_last")
        else:
            pv = ps_pv.tile([DK1, QB], F32, tag="pv")
        cur_sc = []
        for kt in range(NKT):
            ps = ps_a.tile([128, QB], F32, tag="a")
            for u in range(QB // 512):
                nc.tensor.matmul(
                    ps[:, bass.ts(u, 512)],
                    kh_sb[ht][hr:hr + 64, bass.ts(kt, 128)],
                    qh_sb[ht][hr:hr + 64, q0 + u * 512:q0 + (u + 1) * 512],
                    start=True, stop=True,
                )
            sct = sctpool.tile([128, QB], BF16, tag="sct", name="sct")
            nc.scalar.activation(sct[:], ps[:], EXPF, scale=0.125)
            cur_sc.append(sct)
            if prev is not None:
                ppv, psc, ph, pqb = prev
                for u in range(QB // 512):
                    nc.tensor.matmul(
                        ppv[:, bass.ts(u, 512)],
                        v_sb[kt][:, ph * DK1:(ph + 1) * DK1],
                        psc[kt][:, bass.ts(u, 512)],
                        start=(kt == 0), stop=(kt == NKT - 1),
                    )
            if bi == last and kt >= 1:
                # self-interleave: the last block folds its own PV in with
                # a one-slot lag so the drain after the loop is only kt=15.
                for u in range(QB // 512):
                    nc.tensor.matmul(
                        pv[:, bass.ts(u, 512)],
                        v_sb[kt - 1][:, h * DK1:(h + 1) * DK1],
                        cur_sc[kt - 1][:, bass.ts(u, 512)],
                        start=(kt - 1 == 0), stop=False,
                    )
        if prev is not None:
            norm_and_out(prev[0], prev[2], prev[3])
        prev = (pv, cur_sc, h, qb)

    # drain: the last block only needs kt=15
    ppv, psc, ph, pqb = prev
    for u in range(QB // 512):
        nc.tensor.matmul(
            ppv[:, bass.ts(u, 512)],
            v_sb[NKT - 1][:, ph * DK1:(ph + 1) * DK1],
            psc[NKT - 1][:, bass.ts(u, 512)],
            start=False, stop=True,
        )
    norm_and_out(ppv, ph, pqb)
    for c in range(NCH):
        out_proj(c)


def _prep_inputs(q, k, v, Wq, bq, Wk, bk, Wv, bv, Wo, bo):
    """Build the per-core input maps (host-side sharding)."""
    in_maps = []
    for c in range(N_CORES):
        b, hg = c // 4, c % 4
        fsl = slice(FEAT * hg, FEAT * (hg + 1))
        osl = slice(OCOL * hg, OCOL * (hg + 1))
        wv_aug = np.zeros((D, VW), np.float32)
        bv_aug = np.zeros((VW,), np.float32)
        for h in range(HPC):
            rows = slice(FEAT * hg + DK * h, FEAT * hg + DK * (h + 1))
            wv_aug[:, h * DK1:h * DK1 + DK] = Wv[rows, :].T
            bv_aug[h * DK1:h * DK1 + DK] = bv[rows]
            bv_aug[h * DK1 + DK] = 1.0
        in_maps.append({
            "xq": np.ascontiguousarray(q[b].T).astype(NPBF16),
            "xk": np.ascontiguousarray(k[b].T).astype(NPBF16),
            "xv": np.ascontiguousarray(v[b].T).astype(NPBF16),
            "wq": np.ascontiguousarray(Wq[fsl].T).astype(NPBF16),
            "wk": np.ascontiguousarray(Wk[fsl].T).astype(NPBF16),
            "wv": wv_aug.astype(NPBF16),
            "wo": np.ascontiguousarray(Wo[osl].T).astype(NPBF16),
            "bq": np.ascontiguousarray(
                bq[fsl].reshape(2, 128).T).astype(np.float32),
            "bk": np.ascontiguousarray(
                bk[fsl].reshape(2, 128).T).astype(np.float32),
            "bv": bv_aug.reshape(1, VW).astype(NPBF16),
            "bo": np.ascontiguousarray(
                bo[osl].reshape(1, OCOL)).astype(NPBF16),
        })
    return in_maps


def run_sharded(in_maps, trace=False):
    nc = _build_program()
    res = run_bass_kernel_spmd(nc, in_maps, list(range(N_CORES)), trace=trace)
    full = np.empty((B, S, D), np.float32)
    for c in range(N_CORES):
        b, hg = c // 4, c % 4
        full[b, :, OCOL * hg:OCOL * (hg + 1)] = (
            res.results[c]["out"].astype(np.float32))
    return full, res


def kernel(q, k, v, Wq, bq, Wk, bk, Wv, bv, Wo, bo):
    args = [np.asarray(x, np.float32) for x in
            (q, k, v, Wq, bq, Wk, bk, Wv, bv, Wo, bo)]
    in_maps = _prep_inputs(*args)
    full, _ = run_sharded(in_maps)
    return full
